# revision 8
# baseline (speedup 1.0000x reference)
"""GCN (ConvGraphNet) Trainium2 kernel — 8 NeuronCores, graph/data-parallel.

Strategy (matches sharding_hint: partition nodes across cores):
  - Nodes are sharded row-wise: core c owns real nodes [c*6250, (c+1)*6250),
    stored in a padded slot space of 6272 rows/core (49 tiles of 128).
  - Per GCN layer h_l = relu(A_hat @ (h W) + b) we aggregate on the cheaper
    side: layer 1 aggregates x (table replicated, no comm), layers 2-4
    aggregate p_l = h_{l-1} W_l (computed row-sharded, then AllGather
    replicates the table), layer 5 aggregates h_4 (then applies W5).
  - The gather h[src] uses the custom dma_gather (int16 indices; the 50176
    slot space is split at 32768 into lo/hi table views).
  - The scatter-add (segment-sum by dst) is a TensorE matmul with a host-built
    one-hot matrix S per 128-edge chunk: S[e, dst_local] = norm_e (the
    D^-1/2 (A+I) D^-1/2 edge weight). Aggregation output is produced
    transposed ([D, 128 rows]) so bias+relu are per-partition and the next
    matmul consumes it as lhsT directly.
  - bf16 storage/matmuls, fp32 PSUM accumulation.
"""
import os
import numpy as np

N_CORES = 8
N_NODES = 50000
D_IN = 128
DIMS = [128, 1024, 512, 256, 64, 1]
N_LOC_REAL = N_NODES // N_CORES          # 6250
TILES = 49                               # ceil(6250/128)
N_LOC = TILES * 128                      # 6272
N_PAD = N_CORES * N_LOC                  # 50176
SPLIT = 32768

# per-stage: table width (padded), used width, tiles per gather-group
STAGE_D = [128, 512, 256, 128, 128]
STAGE_U = [128, 512, 256, 64, 64]
TPG = [8, 2, 4, 8, 8]

_CACHE = {}


def _preprocess(edge_index):
    """Build per-core gather-index and S (one-hot scatter) arrays.

    Returns (NL_t, NH_t, idx_lo[8], idx_hi[8], S[8]) where NL_t/NH_t are the
    uniform (max over cores) lo/hi chunk counts per tile.
    """
    src = edge_index[0].astype(np.int64)
    dst = edge_index[1].astype(np.int64)
    loop = np.arange(N_NODES, dtype=np.int64)
    src = np.concatenate([src, loop])
    dst = np.concatenate([dst, loop])
    deg = np.bincount(dst, minlength=N_NODES).astype(np.float64)
    dinv = 1.0 / np.sqrt(np.maximum(deg, 1.0))
    w = (dinv[src] * dinv[dst]).astype(np.float32)

    core = dst // N_LOC_REAL
    dst_local = dst % N_LOC_REAL
    tile_g = dst_local // 128
    dst_in_tile = dst_local % 128
    sslot = (src // N_LOC_REAL) * N_LOC + (src % N_LOC_REAL)
    kind = (sslot >= SPLIT).astype(np.int64)
    idxval = np.where(kind == 0, sslot, sslot - SPLIT)

    key = (core * TILES + tile_g) * 2 + kind
    counts = np.bincount(key, minlength=N_CORES * TILES * 2).reshape(N_CORES, TILES, 2)
    NL_t = np.maximum(np.ceil(counts[:, :, 0] / 128).astype(np.int64).max(axis=0), 1)
    NH_t = np.maximum(np.ceil(counts[:, :, 1] / 128).astype(np.int64).max(axis=0), 1)
    CLo = int(NL_t.sum())
    CHi = int(NH_t.sum())
    Lbase = np.concatenate([[0], np.cumsum(NL_t)])[:-1]      # per tile, lo chunk space
    Hbase = np.concatenate([[0], np.cumsum(NH_t)])[:-1]      # per tile, hi chunk space

    # position of each edge within its (core, tile, kind) segment
    order = np.argsort(key, kind="stable")
    seg_sizes = counts.reshape(-1)
    seg_starts = np.concatenate([[0], np.cumsum(seg_sizes)])[:-1]
    pos_sorted = np.arange(len(src)) - seg_starts[key[order]]
    pos = np.empty(len(src), dtype=np.int64)
    pos[order] = pos_sorted

    # chunk index (within the tile's lo/hi chunk budget) and lane 0..127
    chunk_in_tile = pos // 128
    lane = pos % 128
    # global chunk index in the lo or hi space
    gchunk = np.where(kind == 0, Lbase[tile_g] + chunk_in_tile,
                      Hbase[tile_g] + chunk_in_tile)

    import ml_dtypes
    idx_lo = []
    idx_hi = []
    S_arr = []
    SW = (CLo + CHi) * 128
    for c in range(N_CORES):
        m = core == c
        il = np.zeros((16, CLo * 8), dtype=np.int16)
        ih = np.zeros((16, CHi * 8), dtype=np.int16)
        for arr, sel in ((il, m & (kind == 0)), (ih, m & (kind == 1))):
            gc = gchunk[sel]
            ln = lane[sel]
            col = gc * 8 + ln // 16
            row = ln % 16
            arr[row, col] = idxval[sel].astype(np.int16)
        idx_lo.append(np.tile(il, (8, 1)))
        idx_hi.append(np.tile(ih, (8, 1)))

        S = np.zeros((128, SW), dtype=np.float32)
        ml = m & (kind == 0)
        mh = m & (kind == 1)
        S[lane[ml], gchunk[ml] * 128 + dst_in_tile[ml]] = w[ml]
        S[lane[mh], (CLo + gchunk[mh]) * 128 + dst_in_tile[mh]] = w[mh]
        S_arr.append(S.astype(ml_dtypes.bfloat16))

    return NL_t, NH_t, idx_lo, idx_hi, S_arr


def _build(NL_t, NH_t):
    import concourse.bacc as bacc
    import concourse.mybir as mybir
    from concourse.tile import TileContext

    CLo = int(NL_t.sum())
    CHi = int(NH_t.sum())
    Lbase = np.concatenate([[0], np.cumsum(NL_t)])
    Hbase = np.concatenate([[0], np.cumsum(NH_t)])

    dt = mybir.dt.bfloat16
    f32 = mybir.dt.float32

    nc = bacc.Bacc("TRN2", target_bir_lowering=False, debug=False,
                   enable_asserts=False, num_devices=N_CORES)

    x_t = nc.dram_tensor("x", [N_PAD, 128], dt, kind="ExternalInput")
    idxlo_d = nc.dram_tensor("idxlo", [128, CLo * 8], mybir.dt.int16, kind="ExternalInput")
    idxhi_d = nc.dram_tensor("idxhi", [128, CHi * 8], mybir.dt.int16, kind="ExternalInput")
    S_d = nc.dram_tensor("S", [128, (CLo + CHi) * 128], dt, kind="ExternalInput")
    w1_d = nc.dram_tensor("w1", [128, 1024], dt, kind="ExternalInput")
    w2_d = nc.dram_tensor("w2", [128, 8 * 512], dt, kind="ExternalInput")
    w3_d = nc.dram_tensor("w3", [128, 4 * 256], dt, kind="ExternalInput")
    w4_d = nc.dram_tensor("w4", [128, 2 * 64], dt, kind="ExternalInput")
    w5_d = nc.dram_tensor("w5", [128, 1], dt, kind="ExternalInput")
    b1_d = nc.dram_tensor("b1", [128, 8], f32, kind="ExternalInput")
    b2_d = nc.dram_tensor("b2", [128, 4], f32, kind="ExternalInput")
    b3_d = nc.dram_tensor("b3", [128, 2], f32, kind="ExternalInput")
    b4_d = nc.dram_tensor("b4", [128, 1], f32, kind="ExternalInput")
    b5_d = nc.dram_tensor("b5", [128, 1], f32, kind="ExternalInput")
    ident_d = nc.dram_tensor("ident", [128, 128], dt, kind="ExternalInput")
    out_d = nc.dram_tensor("out", [N_LOC, 1], f32, kind="ExternalOutput")

    n_stages = int(os.environ.get("GCN_STAGES", "5"))
    cc_dims = [None, 512, 256, 128, 128]
    cc_in = [None] + [
        nc.dram_tensor(f"cc{i + 1}_in", [N_LOC, cc_dims[i]], dt) if i < n_stages else None
        for i in range(1, 5)]
    cc_out = [x_t] + [
        nc.dram_tensor(f"cc{i + 1}_out", [N_PAD, cc_dims[i]], dt, addr_space="Shared")
        if i < n_stages else None
        for i in range(1, 5)]
    dbg_d = None
    if n_stages < 5:
        Dn = [512, 256, 128, 128][n_stages - 1]
        dbg_d = nc.dram_tensor("dbg", [N_LOC, Dn], dt, kind="ExternalOutput")

    with TileContext(nc) as tc:
        with (
            tc.tile_pool(name="const", bufs=1) as constp,
            tc.tile_pool(name="msg", bufs=2) as msgp,
            tc.tile_pool(name="spool", bufs=2) as spool,
            tc.tile_pool(name="work", bufs=3) as workp,
            tc.tile_pool(name="psA", bufs=2, space="PSUM") as psA,
            tc.tile_pool(name="psB", bufs=2, space="PSUM") as psB,
        ):
            # resident constants
            idxlo = constp.tile([128, CLo * 8], mybir.dt.int16, tag="idxlo")
            idxhi = constp.tile([128, CHi * 8], mybir.dt.int16, tag="idxhi")
            nc.sync.dma_start(out=idxlo[:], in_=idxlo_d[:, :])
            nc.sync.dma_start(out=idxhi[:], in_=idxhi_d[:, :])
            w_sb = []
            for wd, tag in ((w1_d, "w1"), (w2_d, "w2"), (w3_d, "w3"), (w4_d, "w4"), (w5_d, "w5")):
                t = constp.tile(list(wd.shape), dt, tag=tag)
                nc.sync.dma_start(out=t[:], in_=wd[:, :])
                w_sb.append(t)
            b_sb = []
            for bd, tag in ((b1_d, "b1"), (b2_d, "b2"), (b3_d, "b3"), (b4_d, "b4"), (b5_d, "b5")):
                t = constp.tile(list(bd.shape), f32, tag=tag)
                nc.sync.dma_start(out=t[:], in_=bd[:, :])
                b_sb.append(t)
            ident = constp.tile([128, 128], dt, tag="ident")
            nc.sync.dma_start(out=ident[:], in_=ident_d[:, :])

            max_tiles = int(os.environ.get("GCN_MAX_TILES", str(TILES)))
            no_agg = bool(int(os.environ.get("GCN_NO_AGG", "0")))
            for st in range(n_stages):
                D = STAGE_D[st]
                U = STAGE_U[st]
                dblks = (U + 127) // 128
                tab = cc_out[st]
                k = TPG[st]
                # max chunks within any group of k tiles (for msg buffer sizing)
                max_lo = max(int(NL_t[t0:t0 + k].sum()) for t0 in range(0, TILES, k))
                max_hi = max(int(NH_t[t0:t0 + k].sum()) for t0 in range(0, TILES, k))

                for t0 in range(0, min(TILES, max_tiles), k):
                    t1 = min(t0 + k, TILES)
                    gl0, gl1 = int(Lbase[t0]), int(Lbase[t1])
                    gh0, gh1 = int(Hbase[t0]), int(Hbase[t1])
                    nlo, nhi = gl1 - gl0, gh1 - gh0

                    msg_lo = msgp.tile([128, max_lo * D], dt, tag="mlo")
                    msg_hi = msgp.tile([128, max_hi * D], dt, tag="mhi")
                    nc.gpsimd.dma_gather(
                        out_ap=msg_lo[:, :nlo * D].rearrange("p (c e) -> p c e", e=D),
                        in_ap=tab[0:SPLIT, :],
                        idxs_ap=idxlo[:, gl0 * 8:gl1 * 8],
                        num_idxs=nlo * 128, num_idxs_reg=nlo * 128,
                        elem_size=D, single_packet=False)
                    nc.gpsimd.dma_gather(
                        out_ap=msg_hi[:, :nhi * D].rearrange("p (c e) -> p c e", e=D),
                        in_ap=tab[SPLIT:N_PAD, :],
                        idxs_ap=idxhi[:, gh0 * 8:gh1 * 8],
                        num_idxs=nhi * 128, num_idxs_reg=nhi * 128,
                        elem_size=D, single_packet=False)
                    # S slab for this group: lo chunks then hi chunks
                    s_sb = spool.tile([128, (max_lo + max_hi) * 128], dt, tag="sslab")
                    nc.sync.dma_start(out=s_sb[:, :nlo * 128], in_=S_d[:, gl0 * 128:gl1 * 128])
                    nc.sync.dma_start(out=s_sb[:, nlo * 128:(nlo + nhi) * 128],
                                      in_=S_d[:, (CLo + gh0) * 128:(CLo + gh1) * 128])

                    if no_agg:
                        continue
                    for t in range(t0, t1):
                        # chunk list: (msg buffer, col position, s col position)
                        chunks = []
                        for j in range(int(NL_t[t])):
                            p = int(Lbase[t]) - gl0 + j
                            chunks.append((msg_lo, p, p))
                        for j in range(int(NH_t[t])):
                            p = int(Hbase[t]) - gh0 + j
                            chunks.append((msg_hi, p, nlo + p))
                        agg = psA.tile([128, dblks * 128], f32, tag="agg")
                        for db in range(dblks):
                            m = min(128, U - db * 128)
                            for j, (buf, p, sp) in enumerate(chunks):
                                nc.tensor.matmul(
                                    agg[0:m, db * 128:db * 128 + 128],
                                    buf[:, p * D + db * 128: p * D + db * 128 + m],
                                    s_sb[:, sp * 128:(sp + 1) * 128],
                                    start=(j == 0), stop=(j == len(chunks) - 1))

                        if st == 0:
                            # stage1: h1_T = relu(W1^T @ agg + b1); p2 = h1 @ W2
                            agg_s = workp.tile([128, 128], dt, tag="aggs")
                            nc.scalar.activation(agg_s[:], agg[:, 0:128],
                                                 mybir.ActivationFunctionType.Copy)
                            z = psB.tile([128, 1024], f32, tag="zp")
                            for ob in range(8):
                                nc.tensor.matmul(z[:, ob * 128:(ob + 1) * 128],
                                                 w_sb[0][:, ob * 128:(ob + 1) * 128],
                                                 agg_s[:], start=True, stop=True)
                            hT = workp.tile([128, 8 * 128], dt, tag="hT")
                            for ob in range(8):
                                nc.scalar.activation(hT[:, ob * 128:(ob + 1) * 128],
                                                     z[:, ob * 128:(ob + 1) * 128],
                                                     mybir.ActivationFunctionType.Relu,
                                                     bias=b_sb[0][:, ob:ob + 1])
                            pp = psA.tile([128, 512], f32, tag="pp")
                            for kk in range(8):
                                nc.tensor.matmul(pp[:, 0:512],
                                                 hT[:, kk * 128:(kk + 1) * 128],
                                                 w_sb[1][:, kk * 512:(kk + 1) * 512],
                                                 start=(kk == 0), stop=(kk == 7))
                            p_sb = workp.tile([128, 512], dt, tag="p")
                            nc.vector.tensor_copy(p_sb[:], pp[:])
                            tgt = cc_in[1] if n_stages > 1 else dbg_d
                            nc.sync.dma_start(out=tgt[t * 128:(t + 1) * 128, :], in_=p_sb[:])

                        elif st in (1, 2):
                            # h = relu(agg + b); p_next = h @ W_next
                            hT = workp.tile([128, dblks * 128], dt, tag="hT")
                            for db in range(dblks):
                                nc.scalar.activation(hT[:, db * 128:(db + 1) * 128],
                                                     agg[:, db * 128:(db + 1) * 128],
                                                     mybir.ActivationFunctionType.Relu,
                                                     bias=b_sb[st][:, db:db + 1])
                            N_out = 256 if st == 1 else 64
                            wnext = w_sb[st + 1]
                            pp = psA.tile([128, 512], f32, tag="pp")
                            for kk in range(dblks):
                                nc.tensor.matmul(pp[:, 0:N_out],
                                                 hT[:, kk * 128:(kk + 1) * 128],
                                                 wnext[:, kk * N_out:(kk + 1) * N_out],
                                                 start=(kk == 0), stop=(kk == dblks - 1))
                            if st == 1:
                                p_sb = workp.tile([128, 256], dt, tag="p")
                                nc.vector.tensor_copy(p_sb[:], pp[:, 0:256])
                                tgt = cc_in[2] if n_stages > 2 else dbg_d
                                nc.sync.dma_start(out=tgt[t * 128:(t + 1) * 128, :], in_=p_sb[:])
                            else:
                                p_sb = workp.tile([128, 128], dt, tag="p")
                                nc.vector.memset(p_sb[:, 64:128], 0.0)
                                nc.vector.tensor_copy(p_sb[:, 0:64], pp[:, 0:64])
                                tgt = cc_in[3] if n_stages > 3 else dbg_d
                                nc.sync.dma_start(out=tgt[t * 128:(t + 1) * 128, :], in_=p_sb[:])

                        elif st == 3:
                            # h4_T = relu(agg + b4) [64,128]; transpose; pad; -> cc5
                            hT = workp.tile([128, 128], dt, tag="hT")
                            nc.scalar.activation(hT[0:64, 0:128], agg[0:64, 0:128],
                                                 mybir.ActivationFunctionType.Relu,
                                                 bias=b_sb[3][0:64, 0:1])
                            tp = psB.tile([128, 128], dt, tag="zp")
                            nc.tensor.transpose(tp[0:128, 0:64], hT[0:64, 0:128], ident[0:64, 0:64])
                            p_sb = workp.tile([128, 128], dt, tag="p")
                            nc.vector.memset(p_sb[:, 64:128], 0.0)
                            nc.vector.tensor_copy(p_sb[:, 0:64], tp[:, 0:64])
                            tgt = cc_in[4] if n_stages > 4 else dbg_d
                            nc.sync.dma_start(out=tgt[t * 128:(t + 1) * 128, :], in_=p_sb[:])

                        else:
                            # stage5: out = agg5 @ W5 + b5
                            agg_s = workp.tile([128, 128], dt, tag="aggs")
                            nc.scalar.activation(agg_s[0:64, :], agg[0:64, 0:128],
                                                 mybir.ActivationFunctionType.Copy)
                            fo = psB.tile([128, 1], f32, tag="zp")
                            nc.tensor.matmul(fo[:, 0:1], agg_s[0:64, :], w_sb[4][0:64, 0:1],
                                             start=True, stop=True)
                            o_sb = workp.tile([128, 1], f32, tag="osb")
                            nc.vector.tensor_add(o_sb[:], fo[:], b_sb[4][:, 0:1])
                            nc.sync.dma_start(out=out_d[t * 128:(t + 1) * 128, :], in_=o_sb[:])

                # AllGather the next table
                if st < n_stages - 1 and st < 4:
                    nc.gpsimd.collective_compute(
                        "AllGather", mybir.AluOpType.bypass,
                        replica_groups=[list(range(N_CORES))],
                        ins=[cc_in[st + 1][:, :]], outs=[cc_out[st + 1][:, :]])

    nc.compile()
    return nc


def _prep_inputs(x, Ws, bs, NL_t, NH_t, idx_lo, idx_hi, S_arr):
    import ml_dtypes
    bf16 = ml_dtypes.bfloat16
    xp = np.zeros((N_PAD, 128), dtype=bf16)
    xs = x.reshape(N_CORES, N_LOC_REAL, 128)
    for c in range(N_CORES):
        xp[c * N_LOC:c * N_LOC + N_LOC_REAL] = xs[c].astype(bf16)

    def chunked(W, kc, n):
        return np.ascontiguousarray(
            W.reshape(kc, 128, n).transpose(1, 0, 2).reshape(128, kc * n)).astype(bf16)

    w1 = Ws[0].astype(bf16)                                   # [128, 1024]
    w2 = chunked(np.asarray(Ws[1]), 8, 512)
    w3 = chunked(np.asarray(Ws[2]), 4, 256)
    w4 = chunked(np.asarray(Ws[3]), 2, 64)
    w5 = np.zeros((128, 1), dtype=bf16)
    w5[0:64] = np.asarray(Ws[4]).astype(bf16)

    def bcol(b, nb):
        a = np.zeros((128, nb), dtype=np.float32)
        bb = np.asarray(b, dtype=np.float32)
        for i in range(nb):
            seg = bb[i * 128:(i + 1) * 128]
            a[:len(seg), i] = seg
        return a

    b1 = bcol(bs[0], 8)
    b2 = bcol(bs[1], 4)
    b3 = bcol(bs[2], 2)
    b4 = bcol(bs[3], 1)
    b5 = np.full((128, 1), np.float32(np.asarray(bs[4])[0]), dtype=np.float32)
    ident = np.eye(128, dtype=bf16)

    in_maps = []
    for c in range(N_CORES):
        in_maps.append({
            "x": xp, "idxlo": idx_lo[c], "idxhi": idx_hi[c], "S": S_arr[c],
            "w1": np.ascontiguousarray(w1), "w2": w2, "w3": w3, "w4": w4, "w5": w5,
            "b1": b1, "b2": b2, "b3": b3, "b4": b4, "b5": b5, "ident": ident,
        })
    return in_maps


def kernel(x, edge_index, Ws, bs):
    from concourse import bass_utils
    x = np.asarray(x, dtype=np.float32)
    edge_index = np.asarray(edge_index)
    key = hash(edge_index.tobytes())
    if key not in _CACHE:
        NL_t, NH_t, idx_lo, idx_hi, S_arr = _preprocess(edge_index)
        nc = _build(NL_t, NH_t)
        _CACHE[key] = (nc, NL_t, NH_t, idx_lo, idx_hi, S_arr)
    nc, NL_t, NH_t, idx_lo, idx_hi, S_arr = _CACHE[key]
    in_maps = _prep_inputs(x, Ws, bs, NL_t, NH_t, idx_lo, idx_hi, S_arr)
    res = bass_utils.run_bass_kernel_spmd(
        nc, in_maps, core_ids=list(range(N_CORES)),
        trace=bool(int(os.environ.get("GCN_TRACE", "0"))))
    kernel.last_results = res
    out = np.empty((N_NODES, 1), dtype=np.float32)
    for c in range(N_CORES):
        out[c * N_LOC_REAL:(c + 1) * N_LOC_REAL] = res.results[c]["out"][:N_LOC_REAL]
    return out


# revision 13
# speedup vs baseline: 1.4384x; 1.4384x over previous
"""GCN (ConvGraphNet) Trainium2 kernel — 8 NeuronCores, graph/data-parallel.

Strategy (matches sharding_hint: partition nodes across cores):
  - Nodes are sharded row-wise: core c owns real nodes [c*6250, (c+1)*6250),
    stored in a padded slot space of 6272 rows/core (49 tiles of 128).
  - Per GCN layer h_l = relu(A_hat @ (h W) + b) we aggregate on the cheaper
    side: layer 1 aggregates x (table replicated, no comm), layers 2-4
    aggregate p_l = h_{l-1} W_l (computed row-sharded, then AllGather
    replicates the table), layer 5 aggregates h_4 (then applies W5).
  - The gather h[src] uses the custom dma_gather (int16 indices; the 50176
    slot space is split at 32768 into lo/hi table views). Edges are packed
    tightly into 128-edge chunks per (tile-group, lo/hi); chunks may straddle
    tile boundaries — each tile's matmuls read a static chunk window and the
    per-core one-hot S content selects its own edges (zeros elsewhere).
  - Self-loop messages are contiguous local rows: plain HWDGE DMA from the
    core-local table (x_loc / cc_in), no gather descriptors at all.
  - The scatter-add (segment-sum by dst) is a TensorE matmul with a host-built
    one-hot matrix S per 128-edge chunk: S[e, dst_local] = norm_e (the
    D^-1/2 (A+I) D^-1/2 edge weight). Aggregation output is produced
    transposed ([D, 128 rows]) so bias+relu are per-partition and the next
    matmul consumes it as lhsT directly.
  - bf16 storage/matmuls, fp32 PSUM accumulation.
"""
import os
import numpy as np

N_CORES = 8
N_NODES = 50000
D_IN = 128
DIMS = [128, 1024, 512, 256, 64, 1]
N_LOC_REAL = N_NODES // N_CORES          # 6250
TILES = 49                               # ceil(6250/128)
N_LOC = TILES * 128                      # 6272
N_PAD = N_CORES * N_LOC                  # 50176
SPLIT = 32768
KTILES = 4                               # tiles per gather group (all stages)

# per-stage: table width (padded), used width
STAGE_D = [128, 512, 256, 128, 128]
STAGE_U = [128, 512, 256, 64, 64]

_CACHE = {}


class Plan:
    pass


def _preprocess(edge_index):
    """Pack edges into tight 128-edge chunks per (group, lo/hi kind).

    Group = KTILES consecutive dst tiles. Within a (core, group, kind) the
    edges are ordered by dst tile and packed with no per-tile alignment;
    chunks straddling tile boundaries are read by both tiles' matmuls (the
    per-core S one-hot selects each tile's edges). Self loops are NOT in the
    edge list (handled as direct DMA chunks with diagonal S blocks).

    Returns a Plan with:
      n_groups, CLo_g/CHi_g (per-group chunk budgets),
      lo_base/hi_base (group chunk offsets), tile windows per kind,
      idx_lo/idx_hi[core], S[core], wself[core] diag blocks.
    """
    import ml_dtypes
    src = edge_index[0].astype(np.int64)
    dst = edge_index[1].astype(np.int64)
    deg = np.bincount(dst, minlength=N_NODES).astype(np.float64) + 1.0  # +self
    dinv = 1.0 / np.sqrt(deg)
    w = (dinv[src] * dinv[dst]).astype(np.float32)
    wself = (dinv * dinv).astype(np.float32)

    core = dst // N_LOC_REAL
    dst_local = dst % N_LOC_REAL
    tile_g = dst_local // 128
    dst_in_tile = dst_local % 128
    grp = tile_g // KTILES
    n_groups = (TILES + KTILES - 1) // KTILES
    sslot = (src // N_LOC_REAL) * N_LOC + (src % N_LOC_REAL)
    kind = (sslot >= SPLIT).astype(np.int64)
    idxval = np.where(kind == 0, sslot, sslot - SPLIT)

    # order edges by (core, group, kind, tile); position within segment
    key = ((core * n_groups + grp) * 2 + kind) * TILES + tile_g
    order = np.lexsort((key,))
    # position within (core, group, kind)
    seg_key = (core * n_groups + grp) * 2 + kind
    counts = np.bincount(seg_key, minlength=N_CORES * n_groups * 2)
    seg_starts = np.concatenate([[0], np.cumsum(counts)])[:-1]
    pos = np.empty(len(src), dtype=np.int64)
    pos[order] = np.arange(len(src)) - seg_starts[seg_key[order]]

    cnt3 = counts.reshape(N_CORES, n_groups, 2)
    CLo_g = np.ceil(cnt3[:, :, 0].max(axis=0) / 128).astype(np.int64)   # per group
    CHi_g = np.ceil(cnt3[:, :, 1].max(axis=0) / 128).astype(np.int64)
    lo_base = np.concatenate([[0], np.cumsum(CLo_g)])                   # [n_groups+1]
    hi_base = np.concatenate([[0], np.cumsum(CHi_g)])
    CLo = int(lo_base[-1])
    CHi = int(hi_base[-1])

    # per-(core,group,kind,tile) start offsets -> static tile chunk windows
    tkey = ((core * n_groups + grp) * 2 + kind) * TILES + tile_g
    tcounts = np.bincount(tkey, minlength=N_CORES * n_groups * 2 * TILES)
    tcounts = tcounts.reshape(N_CORES, n_groups, 2, TILES)
    # cumulative within (core, group, kind) over tiles
    tstart = np.cumsum(tcounts, axis=3) - tcounts
    tend = tstart + tcounts
    win = np.zeros((n_groups, 2, KTILES, 2), dtype=np.int64)  # [g,kind,t_in_g,(c0,c1)]
    for g in range(n_groups):
        for kd in range(2):
            for ti in range(KTILES):
                t = g * KTILES + ti
                if t >= TILES:
                    continue
                s = tstart[:, g, kd, t]
                e = tend[:, g, kd, t]
                c0 = int((s // 128).min())
                c1 = int(np.maximum(np.ceil(e / 128), (s // 128) + 0).max())
                c1 = max(c1, c0)
                win[g, kd, ti] = (c0, c1)

    # S block layout: per tile contiguous [lo-window blocks | hi-window blocks |
    # self block]. Chunks shared between two tiles get one S block PER READER
    # TILE (each holding only that tile's edges).
    s_tile_base = np.zeros(TILES, dtype=np.int64)     # first block of tile t
    s_grp_base = np.zeros(n_groups, dtype=np.int64)   # first block of group g
    acc = 0
    for t in range(TILES):
        g, ti = t // KTILES, t % KTILES
        if ti == 0:
            s_grp_base[g] = acc
        s_tile_base[t] = acc
        acc += int(win[g, 0, ti, 1] - win[g, 0, ti, 0]) + \
               int(win[g, 1, ti, 1] - win[g, 1, ti, 0]) + 1
    SW = acc * 128

    chunk_in_seg = pos // 128
    lane = pos % 128
    gchunk = np.where(kind == 0, lo_base[grp], hi_base[grp]) + chunk_in_seg
    # S block of each edge: tile base + (chunk - window start) [+ lo window for hi]
    ti_in_g = tile_g % KTILES
    win_lo_start = win[grp, 0, ti_in_g, 0]
    win_lo_len = win[grp, 0, ti_in_g, 1] - win_lo_start
    win_hi_start = win[grp, 1, ti_in_g, 0]
    s_block = np.where(
        kind == 0,
        s_tile_base[tile_g] + (chunk_in_seg - win_lo_start),
        s_tile_base[tile_g] + win_lo_len + (chunk_in_seg - win_hi_start))

    idx_lo = []
    idx_hi = []
    S_arr = []
    selfmask = np.arange(128)
    for c in range(N_CORES):
        m = core == c
        il = np.zeros((16, CLo * 8), dtype=np.int16)
        ih = np.zeros((16, CHi * 8), dtype=np.int16)
        for arr, sel in ((il, m & (kind == 0)), (ih, m & (kind == 1))):
            gc = gchunk[sel]
            ln = lane[sel]
            arr[ln % 16, gc * 8 + ln // 16] = idxval[sel].astype(np.int16)
        idx_lo.append(np.tile(il, (8, 1)))
        idx_hi.append(np.tile(ih, (8, 1)))

        S = np.zeros((128, SW), dtype=np.float32)
        S[lane[m], s_block[m] * 128 + dst_in_tile[m]] = w[m]
        # self-loop diagonal block = last block of each tile
        for t in range(TILES):
            g, ti = t // KTILES, t % KTILES
            colbase = int(s_tile_base[t] +
                          (win[g, 0, ti, 1] - win[g, 0, ti, 0]) +
                          (win[g, 1, ti, 1] - win[g, 1, ti, 0])) * 128
            n0 = c * N_LOC_REAL + t * 128
            nreal = min(128, N_LOC_REAL - t * 128)
            S[selfmask[:nreal], colbase + selfmask[:nreal]] = wself[n0:n0 + nreal]
        S_arr.append(S.astype(ml_dtypes.bfloat16))

    pl = Plan()
    pl.n_groups = n_groups
    pl.CLo_g, pl.CHi_g = CLo_g, CHi_g
    pl.lo_base, pl.hi_base = lo_base, hi_base
    pl.CLo, pl.CHi = CLo, CHi
    pl.s_tile_base, pl.s_grp_base = s_tile_base, s_grp_base
    pl.n_sblocks = acc
    pl.SW = SW
    pl.win = win
    pl.idx_lo, pl.idx_hi, pl.S = idx_lo, idx_hi, S_arr
    return pl


def _build(pl):
    import concourse.bacc as bacc
    import concourse.mybir as mybir
    from concourse.tile import TileContext

    dt = mybir.dt.bfloat16
    f32 = mybir.dt.float32

    nc = bacc.Bacc("TRN2", target_bir_lowering=False, debug=False,
                   enable_asserts=False, num_devices=N_CORES)

    x_t = nc.dram_tensor("x", [N_PAD, 128], dt, kind="ExternalInput")
    x_loc = nc.dram_tensor("x_loc", [N_LOC, 128], dt, kind="ExternalInput")
    idxlo_d = nc.dram_tensor("idxlo", [128, pl.CLo * 8], mybir.dt.int16, kind="ExternalInput")
    idxhi_d = nc.dram_tensor("idxhi", [128, pl.CHi * 8], mybir.dt.int16, kind="ExternalInput")
    S_d = nc.dram_tensor("S", [128, pl.SW], dt, kind="ExternalInput")
    w1_d = nc.dram_tensor("w1", [128, 1024], dt, kind="ExternalInput")
    w2_d = nc.dram_tensor("w2", [128, 8 * 512], dt, kind="ExternalInput")
    w3_d = nc.dram_tensor("w3", [128, 4 * 256], dt, kind="ExternalInput")
    w4_d = nc.dram_tensor("w4", [128, 2 * 64], dt, kind="ExternalInput")
    w5_d = nc.dram_tensor("w5", [128, 1], dt, kind="ExternalInput")
    b1_d = nc.dram_tensor("b1", [128, 8], f32, kind="ExternalInput")
    b2_d = nc.dram_tensor("b2", [128, 4], f32, kind="ExternalInput")
    b3_d = nc.dram_tensor("b3", [128, 2], f32, kind="ExternalInput")
    b4_d = nc.dram_tensor("b4", [128, 1], f32, kind="ExternalInput")
    b5_d = nc.dram_tensor("b5", [128, 1], f32, kind="ExternalInput")
    ident_d = nc.dram_tensor("ident", [128, 128], dt, kind="ExternalInput")
    out_d = nc.dram_tensor("out", [N_LOC, 1], f32, kind="ExternalOutput")

    n_stages = int(os.environ.get("GCN_STAGES", "5"))
    cc_dims = [None, 512, 256, 128, 128]
    cc_in = [None] + [
        nc.dram_tensor(f"cc{i + 1}_in", [N_LOC, cc_dims[i]], dt) if i < n_stages else None
        for i in range(1, 5)]
    cc_out = [x_t] + [
        nc.dram_tensor(f"cc{i + 1}_out", [N_PAD, cc_dims[i]], dt, addr_space="Shared")
        if i < n_stages else None
        for i in range(1, 5)]
    loc_tab = [x_loc] + cc_in[1:]
    dbg_d = None
    if n_stages < 5:
        Dn = [512, 256, 128, 128][n_stages - 1]
        dbg_d = nc.dram_tensor("dbg", [N_LOC, Dn], dt, kind="ExternalOutput")

    max_clo = int(pl.CLo_g.max())
    max_chi = int(pl.CHi_g.max())
    grp_blocks = [
        (int(pl.s_grp_base[g + 1]) if g + 1 < pl.n_groups else pl.n_sblocks)
        - int(pl.s_grp_base[g])
        for g in range(pl.n_groups)]
    max_slab = max(grp_blocks)

    with TileContext(nc) as tc:
        with (
            tc.tile_pool(name="const", bufs=1) as constp,
            tc.tile_pool(name="msg", bufs=2) as msgp,
            tc.tile_pool(name="spool", bufs=2) as spool,
            tc.tile_pool(name="work", bufs=3) as workp,
            tc.tile_pool(name="psA", bufs=2, space="PSUM") as psA,
            tc.tile_pool(name="psB", bufs=2, space="PSUM") as psB,
        ):
            # resident constants
            idxlo = constp.tile([128, pl.CLo * 8], mybir.dt.int16, tag="idxlo")
            idxhi = constp.tile([128, pl.CHi * 8], mybir.dt.int16, tag="idxhi")
            nc.sync.dma_start(out=idxlo[:], in_=idxlo_d[:, :])
            nc.sync.dma_start(out=idxhi[:], in_=idxhi_d[:, :])
            w_sb = []
            for wd, tag in ((w1_d, "w1"), (w2_d, "w2"), (w3_d, "w3"), (w4_d, "w4"), (w5_d, "w5")):
                t = constp.tile(list(wd.shape), dt, tag=tag)
                nc.sync.dma_start(out=t[:], in_=wd[:, :])
                w_sb.append(t)
            b_sb = []
            for bd, tag in ((b1_d, "b1"), (b2_d, "b2"), (b3_d, "b3"), (b4_d, "b4"), (b5_d, "b5")):
                t = constp.tile(list(bd.shape), f32, tag=tag)
                nc.sync.dma_start(out=t[:], in_=bd[:, :])
                b_sb.append(t)
            ident = constp.tile([128, 128], dt, tag="ident")
            nc.sync.dma_start(out=ident[:], in_=ident_d[:, :])

            for st in range(n_stages):
                D = STAGE_D[st]
                U = STAGE_U[st]
                dblks = (U + 127) // 128

                for g in range(pl.n_groups):
                    kt = min(KTILES, TILES - g * KTILES)
                    nlo = int(pl.CLo_g[g])
                    nhi = int(pl.CHi_g[g])
                    gl0 = int(pl.lo_base[g])
                    gh0 = int(pl.hi_base[g])
                    # msg layout: [lo chunks | hi chunks | self chunks]
                    ntot = nlo + nhi + kt
                    msg = msgp.tile([128, (max_clo + max_chi + KTILES) * D], dt, tag="msg")
                    nc.gpsimd.dma_gather(
                        out_ap=msg[:, :nlo * D].rearrange("p (c e) -> p c e", e=D),
                        in_ap=cc_out[st][0:SPLIT, :],
                        idxs_ap=idxlo[:, gl0 * 8:(gl0 + nlo) * 8],
                        num_idxs=nlo * 128, num_idxs_reg=nlo * 128,
                        elem_size=D, single_packet=False)
                    nc.gpsimd.dma_gather(
                        out_ap=msg[:, nlo * D:(nlo + nhi) * D].rearrange("p (c e) -> p c e", e=D),
                        in_ap=cc_out[st][SPLIT:N_PAD, :],
                        idxs_ap=idxhi[:, gh0 * 8:(gh0 + nhi) * 8],
                        num_idxs=nhi * 128, num_idxs_reg=nhi * 128,
                        elem_size=D, single_packet=False)
                    # self chunks: plain DMA of local rows
                    for ti in range(kt):
                        t = g * KTILES + ti
                        nc.sync.dma_start(
                            out=msg[:, (nlo + nhi + ti) * D:(nlo + nhi + ti + 1) * D],
                            in_=loc_tab[st][t * 128:(t + 1) * 128, :])
    # S slab: per-tile contiguous blocks [lo-window | hi-window | self]
                    scol0 = int(pl.s_grp_base[g])
                    nblk = grp_blocks[g]
                    s_sb = spool.tile([128, max_slab * 128], dt, tag="sslab")
                    nc.sync.dma_start(out=s_sb[:, :nblk * 128],
                                      in_=S_d[:, scol0 * 128:(scol0 + nblk) * 128])

                    for ti in range(kt):
                        t = g * KTILES + ti
                        sbase = int(pl.s_tile_base[t]) - scol0
                        # chunk list: (msg position, s position)
                        chunks = []
                        c0, c1 = pl.win[g, 0, ti]
                        for cc in range(int(c0), int(c1)):
                            chunks.append((cc, sbase + len(chunks)))
                        c0, c1 = pl.win[g, 1, ti]
                        for cc in range(int(c0), int(c1)):
                            chunks.append((nlo + cc, sbase + len(chunks)))
                        chunks.append((nlo + nhi + ti, sbase + len(chunks)))

                        agg = psA.tile([128, dblks * 128], f32, tag="agg")
                        for db in range(dblks):
                            m = min(128, U - db * 128)
                            for j, (mp, sp) in enumerate(chunks):
                                nc.tensor.matmul(
                                    agg[0:m, db * 128:db * 128 + 128],
                                    msg[:, mp * D + db * 128: mp * D + db * 128 + m],
                                    s_sb[:, sp * 128:(sp + 1) * 128],
                                    start=(j == 0), stop=(j == len(chunks) - 1))

                        if st == 0:
                            # stage1: h1_T = relu(W1^T @ agg + b1); p2 = h1 @ W2
                            agg_s = workp.tile([128, 128], dt, tag="aggs")
                            nc.scalar.activation(agg_s[:], agg[:, 0:128],
                                                 mybir.ActivationFunctionType.Copy)
                            z = psB.tile([128, 1024], f32, tag="zp")
                            for ob in range(8):
                                nc.tensor.matmul(z[:, ob * 128:(ob + 1) * 128],
                                                 w_sb[0][:, ob * 128:(ob + 1) * 128],
                                                 agg_s[:], start=True, stop=True)
                            hT = workp.tile([128, 8 * 128], dt, tag="hT")
                            for ob in range(8):
                                nc.scalar.activation(hT[:, ob * 128:(ob + 1) * 128],
                                                     z[:, ob * 128:(ob + 1) * 128],
                                                     mybir.ActivationFunctionType.Relu,
                                                     bias=b_sb[0][:, ob:ob + 1])
                            pp = psA.tile([128, 512], f32, tag="pp")
                            for kk in range(8):
                                nc.tensor.matmul(pp[:, 0:512],
                                                 hT[:, kk * 128:(kk + 1) * 128],
                                                 w_sb[1][:, kk * 512:(kk + 1) * 512],
                                                 start=(kk == 0), stop=(kk == 7))
                            p_sb = workp.tile([128, 512], dt, tag="p")
                            nc.vector.tensor_copy(p_sb[:], pp[:])
                            tgt = cc_in[1] if n_stages > 1 else dbg_d
                            nc.sync.dma_start(out=tgt[t * 128:(t + 1) * 128, :], in_=p_sb[:])

                        elif st in (1, 2):
                            hT = workp.tile([128, dblks * 128], dt, tag="hT")
                            for db in range(dblks):
                                nc.scalar.activation(hT[:, db * 128:(db + 1) * 128],
                                                     agg[:, db * 128:(db + 1) * 128],
                                                     mybir.ActivationFunctionType.Relu,
                                                     bias=b_sb[st][:, db:db + 1])
                            N_out = 256 if st == 1 else 64
                            wnext = w_sb[st + 1]
                            pp = psA.tile([128, 512], f32, tag="pp")
                            for kk in range(dblks):
                                nc.tensor.matmul(pp[:, 0:N_out],
                                                 hT[:, kk * 128:(kk + 1) * 128],
                                                 wnext[:, kk * N_out:(kk + 1) * N_out],
                                                 start=(kk == 0), stop=(kk == dblks - 1))
                            if st == 1:
                                p_sb = workp.tile([128, 256], dt, tag="p")
                                nc.vector.tensor_copy(p_sb[:], pp[:, 0:256])
                                tgt = cc_in[2] if n_stages > 2 else dbg_d
                                nc.sync.dma_start(out=tgt[t * 128:(t + 1) * 128, :], in_=p_sb[:])
                            else:
                                p_sb = workp.tile([128, 128], dt, tag="p")
                                nc.vector.memset(p_sb[:, 64:128], 0.0)
                                nc.vector.tensor_copy(p_sb[:, 0:64], pp[:, 0:64])
                                tgt = cc_in[3] if n_stages > 3 else dbg_d
                                nc.sync.dma_start(out=tgt[t * 128:(t + 1) * 128, :], in_=p_sb[:])

                        elif st == 3:
                            hT = workp.tile([128, 128], dt, tag="hT")
                            nc.scalar.activation(hT[0:64, 0:128], agg[0:64, 0:128],
                                                 mybir.ActivationFunctionType.Relu,
                                                 bias=b_sb[3][0:64, 0:1])
                            tp = psB.tile([128, 128], dt, tag="zp")
                            nc.tensor.transpose(tp[0:128, 0:64], hT[0:64, 0:128],
                                                ident[0:64, 0:64])
                            p_sb = workp.tile([128, 128], dt, tag="p")
                            nc.vector.memset(p_sb[:, 64:128], 0.0)
                            nc.vector.tensor_copy(p_sb[:, 0:64], tp[:, 0:64])
                            tgt = cc_in[4] if n_stages > 4 else dbg_d
                            nc.sync.dma_start(out=tgt[t * 128:(t + 1) * 128, :], in_=p_sb[:])

                        else:
                            agg_s = workp.tile([128, 128], dt, tag="aggs")
                            nc.scalar.activation(agg_s[0:64, :], agg[0:64, 0:128],
                                                 mybir.ActivationFunctionType.Copy)
                            fo = psB.tile([128, 1], f32, tag="zp")
                            nc.tensor.matmul(fo[:, 0:1], agg_s[0:64, :], w_sb[4][0:64, 0:1],
                                             start=True, stop=True)
                            o_sb = workp.tile([128, 1], f32, tag="osb")
                            nc.vector.tensor_add(o_sb[:], fo[:], b_sb[4][:, 0:1])
                            nc.sync.dma_start(out=out_d[t * 128:(t + 1) * 128, :], in_=o_sb[:])

                # AllGather the next table
                if st < n_stages - 1 and st < 4:
                    nc.gpsimd.collective_compute(
                        "AllGather", mybir.AluOpType.bypass,
                        replica_groups=[list(range(N_CORES))],
                        ins=[cc_in[st + 1][:, :]], outs=[cc_out[st + 1][:, :]])

    nc.compile()
    return nc


def _prep_inputs(x, Ws, bs, pl):
    import ml_dtypes
    bf16 = ml_dtypes.bfloat16
    xp = np.zeros((N_PAD, 128), dtype=bf16)
    xs = np.asarray(x).reshape(N_CORES, N_LOC_REAL, 128)
    for c in range(N_CORES):
        xp[c * N_LOC:c * N_LOC + N_LOC_REAL] = xs[c].astype(bf16)

    def chunked(W, kc, n):
        return np.ascontiguousarray(
            np.asarray(W).reshape(kc, 128, n).transpose(1, 0, 2).reshape(128, kc * n)).astype(bf16)

    w1 = np.ascontiguousarray(np.asarray(Ws[0]).astype(bf16))
    w2 = chunked(Ws[1], 8, 512)
    w3 = chunked(Ws[2], 4, 256)
    w4 = chunked(Ws[3], 2, 64)
    w5 = np.zeros((128, 1), dtype=bf16)
    w5[0:64] = np.asarray(Ws[4]).astype(bf16)

    def bcol(b, nb):
        a = np.zeros((128, nb), dtype=np.float32)
        bb = np.asarray(b, dtype=np.float32)
        for i in range(nb):
            seg = bb[i * 128:(i + 1) * 128]
            a[:len(seg), i] = seg
        return a

    b1 = bcol(bs[0], 8)
    b2 = bcol(bs[1], 4)
    b3 = bcol(bs[2], 2)
    b4 = bcol(bs[3], 1)
    b5 = np.full((128, 1), np.float32(np.asarray(bs[4]).reshape(-1)[0]), dtype=np.float32)
    ident = np.eye(128, dtype=bf16)

    in_maps = []
    for c in range(N_CORES):
        in_maps.append({
            "x": xp, "x_loc": np.ascontiguousarray(xp[c * N_LOC:(c + 1) * N_LOC]),
            "idxlo": pl.idx_lo[c], "idxhi": pl.idx_hi[c], "S": pl.S[c],
            "w1": w1, "w2": w2, "w3": w3, "w4": w4, "w5": w5,
            "b1": b1, "b2": b2, "b3": b3, "b4": b4, "b5": b5, "ident": ident,
        })
    return in_maps


def kernel(x, edge_index, Ws, bs):
    from concourse import bass_utils
    x = np.asarray(x, dtype=np.float32)
    edge_index = np.asarray(edge_index)
    key = hash(edge_index.tobytes())
    if key not in _CACHE:
        pl = _preprocess(edge_index)
        nc = _build(pl)
        _CACHE[key] = (nc, pl)
    nc, pl = _CACHE[key]
    in_maps = _prep_inputs(x, Ws, bs, pl)
    res = bass_utils.run_bass_kernel_spmd(
        nc, in_maps, core_ids=list(range(N_CORES)),
        trace=bool(int(os.environ.get("GCN_TRACE", "0"))))
    kernel.last_results = res
    out = np.empty((N_NODES, 1), dtype=np.float32)
    for c in range(N_CORES):
        out[c * N_LOC_REAL:(c + 1) * N_LOC_REAL] = res.results[c]["out"][:N_LOC_REAL]
    return out
